# revision 56
# baseline (speedup 1.0000x reference)
"""MoE downsample kernel for 8 TRN2 NeuronCores — top-2-only compute.

The reference output keeps only each sample's top-2 (of 4) experts, so the
kernel computes just those: the gate runs on host (tiny matvec + softmax),
then only selected (sample, expert) conv jobs are dispatched to the device,
roughly halving PE work vs dense all-expert compute.

SPMD constraint: one instruction stream for all 8 cores. The per-queue slot
schedule is derived ONLY from the global expert histogram (n0..n3): every
queue processes floor(n_e/2) two-chunk slots plus (n_e odd) one single-chunk
slot per expert, so all 32 queues (8 cores x 4 PE-quadrant queues) execute
identical tap/matmul sequences. Which physical (sample, chunk) each slot
handles is encoded purely in DATA: the host stages each slot's input rows
into a canonical per-core stripe buffer.

Each strided dilated conv decomposes into k*k tap matmuls (lhsT=[64,64]
weight slice vs strided stripe view) accumulated in PSUM per 512-px chunk.
Four 64x64 tile_position quadrant matmuls run concurrently. Taps are outer,
chunks inner (weight-stationary within a slot) to amortize LDWEIGHTS.
BN + conv-bias + GELU fuse into the ScalarE PSUM eviction; the per-sample
gate weights and top-2 concat run on host.
"""

import numpy as np
import ml_dtypes

KS = [3, 5, 7, 9]
DS = [1, 2, 3, 4]
BN_EPS = 1e-5
B, CIN, H, W = 16, 64, 256, 256
CE = 64
PAD = 16                    # left/top pad (max pad_e); right/bottom needs 15
HP = WP = PAD + 256 + 15    # 287
HO = WO = 128
NCORES = 8
NTAPS = sum(k * k for k in KS)  # 164
NCHUNKS = 32                # 4-output-row chunks per (sample, expert) job

_SLOT_BASE = np.cumsum([0] + [k * k for k in KS]).tolist()
_PADE = [DS[e] * (KS[e] - 1) // 2 for e in range(4)]

_COMPILED = {}


def _rows(e, L):
    """Stripe rows needed for L consecutive chunks of expert e."""
    return 8 * L + DS[e] * (KS[e] - 1) - 1


# experts whose tap column offsets are all even need only the even-column
# plane of the input (d*(k-1)/2 even); others need both parity planes
_NPLANES = [2 if (DS[e] * (KS[e] - 1) // 2) % 2 or DS[e] % 2 else 1
            for e in range(4)]
WH = 144  # half-width columns per parity plane


def _qsched(hist):
    """Per-queue slot list [(e, L)] — identical for all 32 queues.

    Ordered heavy/light alternating to spread ScalarE evictions; bookended
    by L=1 single slots of the lightest expert (fast warmup, short tail)."""
    heavy, light = [], []
    for e in range(4):
        if KS[e] >= 7:   # e2/e3: paired chunks, weight-stationary L=2
            heavy += [(e, 2)] * (hist[e] // 2) + [(e, 1)] * (hist[e] % 2)
        else:            # e0/e1: single-chunk slots — small stripes, deep
            light += [(e, 1)] * hist[e]        # psum ring, cheap evictions
    heavy.sort(key=lambda s: -(KS[s[0]] ** 2 * s[1]))
    light.sort(key=lambda s: -(KS[s[0]] ** 2 * s[1]))
    if not heavy:
        return light
    # distribute lights evenly between heavies; start with two lights
    # (cheap warmup slots covering the first big stripes' DMA latency),
    # end with the lightest
    order = []
    nfront = min(3, len(light))
    for _ in range(nfront):
        order.append(light.pop(0))
    nl = len(light)
    take = [(nl * (i + 1)) // len(heavy) - (nl * i) // len(heavy)
            for i in range(len(heavy))]
    for i, h in enumerate(heavy):
        order.append(h)
        for _ in range(take[i]):
            if light:
                order.append(light.pop(0))
    order += light
    return order


def _tap_offsets(e):
    """Yield (wslot, u, v, co) — co is the padded-coords column offset."""
    k, d = KS[e], DS[e]
    pad = _PADE[e]
    for u in range(k):
        for v in range(k):
            yield _SLOT_BASE[e] + u * k + v, u, v, d * v - pad + PAD


def _build_program(hist):
    import concourse.bass as bass  # noqa: F401
    import concourse.mybir as mybir
    import concourse.tile as tile
    from concourse import bacc
    from contextlib import ExitStack

    sched = _qsched(hist)
    R = sum(_NPLANES[e] * _rows(e, L) for e, L in sched)
    nchunks_core = 4 * sum(L for _, L in sched)

    dt = mybir.dt
    nc = bacc.Bacc("TRN2", target_bir_lowering=False, debug=False,
                   num_devices=NCORES)
    # xs[p] stacks the two rhs-half queues (h=0 on channels 0-63, h=1 on
    # 64-127) so each stripe tile fills with ONE 128-partition dma_start —
    # SWDGE descriptor generation is per-dma_start and was the warmup cap
    xs = nc.dram_tensor("xs", [2, 2 * CIN, R, WH], dt.bfloat16,
                        kind="ExternalInput")
    wt = nc.dram_tensor("wt", [2 * CIN, NTAPS, CE], dt.bfloat16,
                        kind="ExternalInput")
    bnp = nc.dram_tensor("bnp", [2 * CE, 4, 2], dt.float32,
                        kind="ExternalInput")
    ntiles = 2 * sum(L for _, L in sched)
    ys = nc.dram_tensor("ys", [ntiles, 2 * CE, 4, WO], dt.bfloat16,
                        kind="ExternalOutput")

    with tile.TileContext(nc) as tc:
        with ExitStack() as ctx:
            consts = ctx.enter_context(tc.tile_pool(name="consts", bufs=1))
            stripe_pool = ctx.enter_context(tc.tile_pool(name="xp", bufs=6))
            st_pool = ctx.enter_context(tc.tile_pool(name="st", bufs=8))
            psum_pool = ctx.enter_context(
                tc.tile_pool(name="ps", bufs=8, space="PSUM"))

            wtile = consts.tile([128, NTAPS, CE], dt.bfloat16)
            bntile = consts.tile([128, 4, 2], dt.float32)
            # DMA order tuned for fast warmup: bn + first expert's weights +
            # first two slots' stripes, then remaining weights, then the
            # rest of the prefetch window.
            eorder = []
            for e, _ in sched:
                if e not in eorder:
                    eorder.append(e)
            # all input DMAs ride the gpsimd SWDGE path: its descriptors
            # spread across all 16 SDMA engines, unlike the narrow
            # HWDGE (sync/scalar) queues
            def load_bn():
                nc.gpsimd.dma_start(out=bntile[:, :, :], in_=bnp.ap())

            def load_weights(e):
                s0 = _SLOT_BASE[e]
                s1 = s0 + KS[e] * KS[e]
                nc.gpsimd.dma_start(out=wtile[:, s0:s1, :],
                                    in_=wt[:, s0:s1, :])

            roffs = np.cumsum(
                [0] + [_NPLANES[e] * _rows(e, L) for e, L in sched]).tolist()
            PREFETCH = 7

            def load_stripes(j):
                e, L = sched[j]
                rows = _rows(e, L)
                P = _NPLANES[e]
                r0 = roffs[j]
                big = e >= 2
                tiles = []
                for pair in range(2):   # pair p: queues (h=0,p), (h=1,p)
                    t = stripe_pool.tile(
                        [128, P * rows, WH], dt.bfloat16,
                        name="xb" if big else "xw",
                        bufs=6 if big else 10)
                    nc.gpsimd.dma_start(
                        out=t[:, :, :],
                        in_=xs[pair, :, r0:r0 + P * rows, :])
                    tiles.append(t)
                return tiles

            stripes = {}
            wdone = set()
            for jj in range(min(PREFETCH, len(sched))):
                stripes[jj] = load_stripes(jj)
                ej = sched[jj][0]
                if ej not in wdone:
                    load_weights(ej)
                    wdone.add(ej)
                if jj == 0:
                    load_bn()   # bn needed only at the first eviction
            for e in eorder:
                if e not in wdone:
                    load_weights(e)
                    wdone.add(e)

            g = 0
            for j, (e, L) in enumerate(sched):
                if j + PREFETCH < len(sched):
                    stripes[j + PREFETCH] = load_stripes(j + PREFETCH)
                tA, tB = stripes.pop(j)
                d = DS[e]
                rows = _rows(e, L)
                P = _NPLANES[e]
                taps = list(_tap_offsets(e))
                # psum tile pss[h][i]: parts 0-63 <- queue (h,p=0),
                # parts 64-127 <- queue (h,p=1). stripe tiles[p]: parts
                # 0-63 <- queue (h=0,p), parts 64-127 <- queue (h=1,p).
                pss = [[psum_pool.tile([128, 512], dt.float32, name="ps")
                        for i in range(L)] for h in range(2)]
                for t, (wslot, u, v, co) in enumerate(taps):
                    first = t == 0
                    last = t == len(taps) - 1
                    par = (co % 2) if P == 2 else 0
                    cc = co // 2
                    for i in range(L):
                        rlo = par * rows + 8 * i + d * u
                        for q in range(4):
                            h, p = q // 2, q % 2
                            stile = tA if p == 0 else tB
                            nc.tensor.matmul(
                                pss[h][i][p * 64:p * 64 + 64, :],
                                wtile[h * 64:h * 64 + 64, wslot, :],
                                stile[h * 64:h * 64 + 64, rlo:rlo + 7:2,
                                      cc:cc + WO],
                                start=first, stop=last,
                                tile_position=(h * 64, p * 64))
                # evict in psum ring-reuse order (h-outer, i-inner); both
                # halves are the same expert, so one full-width ACT covers
                # the whole tile (partition-parallel: same cost as a half)
                for h in range(2):
                    for i in range(L):
                        ps = pss[h][i]
                        st = st_pool.tile([128, 4, WO], dt.bfloat16)
                        nc.scalar.activation(
                            st[:, :, :],
                            ps[:, :].rearrange("p (a b) -> p a b", a=4),
                            mybir.ActivationFunctionType.Gelu,
                            scale=bntile[:, e, 0:1],
                            bias=bntile[:, e, 1:2])
                        # split each tile's output across both HWDGE queues
                        # concurrently — halves per-tile drain latency and
                        # balances the two narrow queues
                        nc.sync.dma_start(out=ys[g, 0:CE, :, :],
                                          in_=st[0:64, :, :])
                        nc.scalar.dma_start(out=ys[g, CE:2 * CE, :, :],
                                            in_=st[64:128, :, :])
                        g += 1

    nc.compile()
    return nc, sched, roffs, nchunks_core


def _get_program(hist):
    key = tuple(hist)
    if key not in _COMPILED:
        _COMPILED[key] = _build_program(hist)
    return _COMPILED[key]


def _host_gate(x, gate_w, gate_b):
    pooled = x.astype(np.float64).mean(axis=(2, 3)).astype(np.float32)
    logits = pooled @ gate_w.T.astype(np.float32) + gate_b
    z = logits - logits.max(axis=1, keepdims=True)
    ez = np.exp(z.astype(np.float32))
    gates = ez / ez.sum(axis=1, keepdims=True)
    idx = np.argsort(-gates, axis=1, kind="stable")[:, :2]
    wsel = np.take_along_axis(gates, idx, axis=1)
    wsel = wsel / (wsel.sum(axis=1, keepdims=True) + 1e-8)
    return idx, wsel.astype(np.float32)


def _assign_jobs(idx, sched):
    """Deal (sample, rank, chunk) work to the 32 queues.

    Returns per-queue slot job lists: jobs[32][len(sched)] = (b, r, c0)."""
    pools = {e: [] for e in range(4)}   # per expert: list of (b, r)
    for b in range(B):
        for r in range(2):
            pools[idx[b, r]].append((b, r))
    # build pair/single supplies per expert, per the slot mix in sched
    nsing = {e: sum(1 for ee, L in sched if ee == e and L == 1)
             for e in range(4)}
    pairs = {e: [(b, r, c0) for (b, r) in pools[e]
                 for c0 in range(0, NCHUNKS, 2)] for e in range(4)}
    singles = {e: [] for e in range(4)}
    for e in range(4):
        if nsing[e]:
            take = pairs[e][-16 * nsing[e]:]
            pairs[e] = pairs[e][:-16 * nsing[e]]
            for (b, r, c0) in take:
                singles[e] += [(b, r, c0), (b, r, c0 + 1)]
    jobs = [[None] * len(sched) for _ in range(32)]
    pi = {e: 0 for e in range(4)}
    si = {e: 0 for e in range(4)}
    for qq in range(32):
        for j, (e, L) in enumerate(sched):
            if L == 2:
                jobs[qq][j] = pairs[e][pi[e]]
                pi[e] += 1
            else:
                jobs[qq][j] = singles[e][si[e]]
                si[e] += 1
    for e in range(4):
        assert pi[e] == len(pairs[e]) and si[e] == len(singles[e])
    return jobs


def _prep_weights(ws, bs, bn_scale, bn_bias, bn_mean, bn_var):
    bf16 = ml_dtypes.bfloat16
    wt = np.empty((2 * CIN, NTAPS, CE), dtype=bf16)
    for e in range(4):
        k = KS[e]
        w = ws[e].astype(np.float32)  # [CE, CIN, k, k]
        blk = w.transpose(1, 2, 3, 0).reshape(CIN, k * k, CE).astype(bf16)
        wt[:CIN, _SLOT_BASE[e]:_SLOT_BASE[e] + k * k, :] = blk
        wt[CIN:, _SLOT_BASE[e]:_SLOT_BASE[e] + k * k, :] = blk
    inv = (bn_scale / np.sqrt(bn_var + BN_EPS)).astype(np.float32)
    shift = (np.stack(bs) * inv + bn_bias - bn_mean * inv).astype(np.float32)
    bnp = np.stack([inv, shift], axis=1)            # [4, 2, CE]
    bnp = np.ascontiguousarray(bnp.transpose(2, 0, 1))  # [CE, 4, 2]
    bnp = np.concatenate([bnp, bnp], axis=0)        # both partition halves
    return wt, bnp


def run(inputs, trace=False):
    from concourse import bass_utils

    x = np.asarray(inputs["x"], dtype=np.float32)
    ws = [np.asarray(inputs[f"w{i}"], dtype=np.float32) for i in range(4)]
    bs = [np.asarray(inputs[f"b{i}"], dtype=np.float32) for i in range(4)]
    bn_scale = np.asarray(inputs["bn_scale"], dtype=np.float32)
    bn_bias = np.asarray(inputs["bn_bias"], dtype=np.float32)
    bn_mean = np.asarray(inputs["bn_mean"], dtype=np.float32)
    bn_var = np.asarray(inputs["bn_var"], dtype=np.float32)
    gate_w = np.asarray(inputs["gate_w"], dtype=np.float32)
    gate_b = np.asarray(inputs["gate_b"], dtype=np.float32)

    idx, wsel = _host_gate(x, gate_w, gate_b)
    hist = [int(np.sum(idx == e)) for e in range(4)]
    nc, sched, roffs, nchunks_core = _get_program(hist)
    jobs = _assign_jobs(idx, sched)
    wt, bnp = _prep_weights(ws, bs, bn_scale, bn_bias, bn_mean, bn_var)

    bf16 = ml_dtypes.bfloat16
    xpad = np.zeros((B, CIN, HP, WP), dtype=bf16)
    xpad[:, :, PAD:PAD + H, PAD:PAD + W] = x.astype(bf16)
    # parity-split column planes: plane c holds padded cols {c, c+2, ...}
    xpar = np.zeros((2, B, CIN, HP, WH), dtype=bf16)
    xpar[0, :, :, :, :] = xpad[:, :, :, 0::2]
    xpar[1, :, :, :, :143] = xpad[:, :, :, 1::2]

    par0 = [next(_tap_offsets(e))[3] % 2 for e in range(4)]
    R = roffs[-1]
    in_maps = []
    for core in range(NCORES):
        xsb = np.empty((2, 2 * CIN, R, WH), dtype=bf16)
        for q in range(4):
            qq = core * 4 + q
            h, p = q // 2, q % 2
            ch = slice(64 * h, 64 * h + 64)
            for j, (e, L) in enumerate(sched):
                b, r, c0 = jobs[qq][j]
                rows = _rows(e, L)
                P = _NPLANES[e]
                row0 = 8 * c0 + PAD - _PADE[e]
                for par in range(P):
                    plane = par if P == 2 else par0[e]
                    o = roffs[j] + par * rows
                    xsb[p, ch, o:o + rows, :] = (
                        xpar[plane, b, :, row0:row0 + rows, :])
        in_maps.append({"xs": xsb, "wt": wt, "bnp": bnp})

    res = bass_utils.run_bass_kernel_spmd(
        nc, in_maps, core_ids=list(range(NCORES)), trace=trace)

    # assemble output: replay schedule to map ys chunks back
    outf = np.zeros((B, 2 * CE, HO, WO), dtype=np.float32)
    for core in range(NCORES):
        ysb = res.results[core]["ys"]  # [nchunks_core, CE, 4, WO]
        g = 0
        for j, (e, L) in enumerate(sched):
            for h in range(2):
                for i in range(L):
                    for p in range(2):
                        q = 2 * h + p
                        b, r, c0 = jobs[core * 4 + q][j]
                        c = c0 + i
                        outf[b, 64 * r:64 * r + 64, 4 * c:4 * c + 4, :] = \
                            ysb[g, 64 * p:64 * p + 64]
                    g += 1
    for b in range(B):
        outf[b, :CE] *= wsel[b, 0]
        outf[b, CE:] *= wsel[b, 1]
    return outf, res


def kernel(**inputs):
    outf, _ = run(inputs, trace=False)
    return outf
